# revision 17
# baseline (speedup 1.0000x reference)
"""Trainium2 Bass kernel for nn_AdvancedLSTMModel (B=262144, D=512, H=16).

The reference network collapses algebraically:
  - seq_len == 1 with zero initial state => LSTM cell is
      h = sigmoid(o) * tanh(sigmoid(i) * tanh(g)),  gates = x @ W_ih.T + b
    (forget gate f is computed but unused since c0 == 0)
  - softmax over a single timestep == 1, so attention context == h1
  - output = h1 @ fc_w.T + fc_b

Strategy: pure data parallel over 8 NeuronCores (batch sharded 32768 rows
per core). The host pre-transposes each x shard to feature-major layout
[128, 4, 32768] so the device streams x.T tiles directly as the matmul
moving operand (contraction over D on partitions; no on-device transpose).

On-device layout: batch is processed in groups of 2048 rows = 4 blocks of
512. All gate tensors are "block-packed": a [128, 512] tile whose partition
quarter q holds the 32 gate/feature channels of block q. Layer-0 gate
matmuls are 4-way column-tiled (M=32 per block into psum partitions 32q),
layer-1 matmuls are 4-way diagonal-tiled (K=32, M=32 at tile (32q, 32q)),
so ScalarE activations and VectorE multiplies always run on full 128
partitions.
"""

import sys

import numpy as np

import concourse.bass as bass
import concourse.mybir as mybir
import concourse.tile as tile
from concourse.bass_utils import run_bass_kernel_spmd

N_CORES = 8
B, D, H = 262144, 512, 16
RC = B // N_CORES          # rows per core
KCH = D // 128             # 4 contraction chunks
BLK = 512                  # rows per block (matmul moving N / psum bank)
GRP = 4 * BLK              # 2048 rows per group (4 column-tiled blocks)
NGRP = RC // GRP           # 16 groups per core
OUT_DMA_GROUPS = 4         # groups batched per output DMA

# Compute dtype for x / weights / activations (psum accumulation is fp32).
DT_NP = np.float16
DT_MB = mybir.dt.float16
F32 = mybir.dt.float32

AF = mybir.ActivationFunctionType


def _patched_drain_and_barrier(self, tick_clock, wait_clock):
    # The nix walrus only encodes one sync-wait per CTRL (drain) instruction;
    # split the Tile tail-drain's waits across one drain each.
    nc = self.nc
    from concourse.tile import ScopedClock

    drain_inst = nc.sync.drain()
    wait_clock.add_sem_waits(
        drain_inst.ins, ScopedClock({None: tick_clock.global_clock})
    )
    si = drain_inst.ins.sync_info
    if si is not None and si.on_wait and len(si.on_wait) > 1:
        waits = list(si.on_wait)
        si.on_wait = waits[:1]
        for w in waits[1:]:
            d2 = nc.sync.drain()
            d2.ins.sync_info = mybir.SyncInfo(on_wait=[w], on_update=[])
    nc.all_engine_barrier()
    popped = nc._tile_sem_poison_stack.pop()
    assert popped is self._sem_poison
    nc.clear_and_free_semaphores(list(self.sems.allocated().values()))
    nc.all_engine_barrier()


tile.TileContext._drain_and_barrier = _patched_drain_and_barrier

MAX_WAITS_PER_INST = 1


def _split_multi_waits(nc, limit=MAX_WAITS_PER_INST):
    """The nix walrus encodes at most `limit` sync-waits per instruction.
    Hoist excess waits onto preceding same-engine nops (engine queues are
    FIFO, so a nop-carried wait gates the next instruction identically)."""
    f = nc.m.functions[0]
    n_split = 0
    for bb in f.blocks:
        insts = bb.instructions
        out = []
        changed = False
        for inst in insts:
            si = inst.sync_info
            if si is not None and si.on_wait and len(si.on_wait) > limit:
                waits = list(si.on_wait)
                head, tail = waits[:-limit], waits[-limit:]
                for i in range(0, len(head), limit):
                    nop = mybir.InstNoOp(
                        name=nc.get_next_instruction_name(), ins=[], outs=[]
                    )
                    nop.engine = inst.engine
                    nop.sync_info = mybir.SyncInfo(
                        on_wait=head[i:i + limit], on_update=[]
                    )
                    out.append(nop)
                si.on_wait = tail
                changed = True
                n_split += 1
            out.append(inst)
        if changed:
            bb.instructions = out
    return n_split


def build_kernel():
    nc = bass.Bass()
    xt = nc.dram_tensor("xt", [128, KCH, RC], DT_MB, kind="ExternalInput")
    w0 = nc.dram_tensor("w0", [128, 3 * KCH * 128], DT_MB, kind="ExternalInput")
    w1 = nc.dram_tensor("w1", [128, 96], DT_MB, kind="ExternalInput")
    fcw = nc.dram_tensor("fcw", [128, 4], DT_MB, kind="ExternalInput")
    bias = nc.dram_tensor("bias", [128, 8], F32, kind="ExternalInput")
    out = nc.dram_tensor("out", [4, NGRP * BLK], F32, kind="ExternalOutput")

    with tile.TileContext(nc) as tc:
        with (
            tc.tile_pool(name="const", bufs=1) as cpool,
            tc.tile_pool(name="xin", bufs=3) as xpool,
            tc.tile_pool(name="work", bufs=3) as wpool,
            tc.tile_pool(name="outp", bufs=2) as opool,
            tc.tile_pool(name="ps0", bufs=1, space="PSUM") as psp0,
            tc.tile_pool(name="ps1", bufs=1, space="PSUM") as psp1,
            tc.tile_pool(name="ps_fc", bufs=2, space="PSUM") as ps_fc,
        ):
            # startup order: i-pass weights, then the first group's x chunks,
            # then everything else — so the first matmuls start ASAP
            w0_sb = cpool.tile([128, 3 * KCH * 128], DT_MB)
            nc.sync.dma_start(out=w0_sb[:, 0:KCH * 128], in_=w0[:, 0:KCH * 128])
            xg_first = xpool.tile([128, KCH, GRP], DT_MB, tag="xg")
            for k in range(KCH):
                nc.sync.dma_start(out=xg_first[:, k, :], in_=xt[:, k, 0:GRP])
            nc.sync.dma_start(out=w0_sb[:, KCH * 128:3 * KCH * 128],
                              in_=w0[:, KCH * 128:3 * KCH * 128])
            bias_sb = cpool.tile([128, 8], F32)
            nc.sync.dma_start(out=bias_sb[:], in_=bias[:])
            w1_sb = cpool.tile([128, 96], DT_MB)
            nc.sync.dma_start(out=w1_sb[:], in_=w1[:])
            fcw_sb = cpool.tile([128, 4], DT_MB)
            nc.sync.dma_start(out=fcw_sb[:], in_=fcw[:])

            # two-stage software pipeline over groups: stage B (layer 1, fc,
            # out) for group t-1 is emitted before stage A (x DMA, layer 0)
            # for group t, so every engine's FIFO leads with ready work.
            stash = {}
            ob = None

            def stage_a_mm(g):
                if g == 0:
                    xg = xg_first
                else:
                    xg = xpool.tile([128, KCH, GRP], DT_MB, tag="xg")
                    nc.sync.dma_start(out=xg[:],
                                      in_=xt[:, :, g * GRP:(g + 1) * GRP])
                ps = []
                for pi in range(3):  # 0: i, 1: o, 2: g
                    p = psp0.tile([128, BLK], F32, tag=f"l0p{pi}")
                    for k in range(KCH):
                        for b in range(4):
                            off = (pi * KCH + k) * 128 + 32 * b
                            nc.tensor.matmul(
                                p[32 * b:32 * b + 32, :],
                                lhsT=w0_sb[:, off:off + 32],
                                rhs=xg[:, k, BLK * b:BLK * (b + 1)],
                                start=(k == 0),
                                stop=(k == KCH - 1),
                                tile_position=(0, 32 * b),
                            )
                    ps.append(p)
                stash[("ps0", g)] = ps

            def stage_a_act(g):
                ps = stash.pop(("ps0", g))
                io0 = wpool.tile([128, 2 * BLK], DT_MB, tag="io0")
                g0 = wpool.tile([128, BLK], DT_MB, tag="g0")
                # order: sigma(i), tanh(g) first so the DVE c-mul can run
                # under sigma(o); then tanh(c) is ready right on time.
                nc.scalar.activation(io0[:, 0:BLK], ps[0][:], AF.Sigmoid,
                                     bias=bias_sb[:, 0:1])
                nc.scalar.activation(g0[:], ps[2][:], AF.Tanh,
                                     bias=bias_sb[:, 2:3])
                c0 = wpool.tile([128, BLK], DT_MB, tag="c0")
                nc.vector.tensor_mul(c0[:], io0[:, 0:BLK], g0[:])
                nc.scalar.activation(io0[:, BLK:2 * BLK], ps[1][:], AF.Sigmoid,
                                     bias=bias_sb[:, 1:2])
                tc0 = wpool.tile([128, BLK], DT_MB, tag="tc0")
                nc.scalar.activation(tc0[:], c0[:], AF.Tanh,
                                     bias=bias_sb[:, 7:8])
                h0 = wpool.tile([128, BLK], DT_MB, tag="h0")
                nc.vector.tensor_mul(h0[:], io0[:, BLK:2 * BLK], tc0[:])
                stash[("h0", g)] = h0

            def stage_b_mm(g):
                h0 = stash.pop(("h0", g))
                ps = []
                for pi in range(3):
                    p = psp1.tile([128, BLK], F32, tag=f"l1p{pi}")
                    for b in range(4):
                        nc.tensor.matmul(
                            p[32 * b:32 * b + 32, :],
                            lhsT=w1_sb[32 * b:32 * b + 32, 32 * pi:32 * pi + 32],
                            rhs=h0[32 * b:32 * b + 32, :],
                            start=True,
                            stop=True,
                            tile_position=(32 * b, 32 * b),
                        )
                    ps.append(p)
                stash[("ps1", g)] = ps

            def stage_b_act(g):
                ps = stash.pop(("ps1", g))
                io1 = wpool.tile([128, 2 * BLK], DT_MB, tag="io1")
                g1 = wpool.tile([128, BLK], DT_MB, tag="g1")
                nc.scalar.activation(io1[:, 0:BLK], ps[0][:], AF.Sigmoid,
                                     bias=bias_sb[:, 3:4])
                nc.scalar.activation(g1[:], ps[2][:], AF.Tanh,
                                     bias=bias_sb[:, 5:6])
                c1 = wpool.tile([128, BLK], DT_MB, tag="c1")
                nc.vector.tensor_mul(c1[:], io1[:, 0:BLK], g1[:])
                nc.scalar.activation(io1[:, BLK:2 * BLK], ps[1][:], AF.Sigmoid,
                                     bias=bias_sb[:, 4:5])
                tc1 = wpool.tile([128, BLK], DT_MB, tag="tc1")
                nc.scalar.activation(tc1[:], c1[:], AF.Tanh,
                                     bias=bias_sb[:, 7:8])
                h1 = wpool.tile([128, BLK], DT_MB, tag="h1")
                nc.vector.tensor_mul(h1[:], io1[:, BLK:2 * BLK], tc1[:])
                stash[("h1", g)] = h1

            def stage_fc(g):
                nonlocal ob
                h1 = stash.pop(("h1", g))
                pf = ps_fc.tile([4, BLK], F32, tag="fc")
                nc.tensor.matmul(pf[:], lhsT=fcw_sb[:, 0:4], rhs=h1[:],
                                 start=True, stop=True, tile_position=(0, 0))
                if g % OUT_DMA_GROUPS == 0:
                    ob = opool.tile([4, OUT_DMA_GROUPS * BLK], F32, tag="ob")
                go = g % OUT_DMA_GROUPS
                # fc bias-add + psum evacuation on the (idle) vector engine
                nc.vector.tensor_scalar_add(ob[:, go * BLK:(go + 1) * BLK],
                                            pf[:], bias_sb[0:4, 6:7])
                if go == OUT_DMA_GROUPS - 1:
                    j = g // OUT_DMA_GROUPS
                    w = OUT_DMA_GROUPS * BLK
                    nc.sync.dma_start(out=out[:, j * w:(j + 1) * w], in_=ob[:])

            # slot t: B-MM(t-1) | A-acts(t) | A-MM(t+1) | B-acts(t-1) | fc(t-1)
            # A-acts(t) lead the ACT FIFO already-ready (their matmuls ran
            # last slot); by the time they finish, B-MM(t-1) has long drained
            # so B-acts(t-1) follow without a gap. The fc matmul (gated on the
            # full act chain) sits last in the PE FIFO behind ready work.
            stage_a_mm(0)
            for t in range(0, NGRP + 1):
                if t >= 1:
                    stage_b_mm(t - 1)
                if t < NGRP:
                    stage_a_act(t)
                if t + 1 < NGRP:
                    stage_a_mm(t + 1)
                if t >= 1:
                    stage_b_act(t - 1)
                    stage_fc(t - 1)
    _split_multi_waits(nc)
    return nc


def _prep_shared(wf0, bf0, wb0, bb0, wf1, bf1, wb1, bb1, attn_w, attn_b,
                 fc_w, fc_b):
    """Build the replicated weight/bias arrays in device layout."""
    # torch LSTM gate row order within [4H]: i, f, g, o
    def rows(w, which):
        s = {"i": 0, "g": 2 * H, "o": 3 * H}[which]
        return w[s:s + H]

    # layer 0 stationary: [128(d), 3(pass), KCH, 128(4 x 32 dup)]
    w0_host = np.zeros((128, 3, KCH, 128), np.float32)
    for pi, which in enumerate(("i", "o", "g")):
        wp = np.concatenate([rows(wf0, which), rows(wb0, which)], axis=0)  # [32, D]
        for k in range(KCH):
            blk = wp[:, 128 * k:128 * (k + 1)].T  # [128(d), 32]
            for b in range(4):
                w0_host[:, pi, k, 32 * b:32 * (b + 1)] = blk
    w0_host = w0_host.reshape(128, 3 * KCH * 128).astype(DT_NP)

    # layer 1 stationary: [128(p = 32-dup'd input), 3*32(out)]
    w1_host = np.zeros((128, 96), np.float32)
    for pi, which in enumerate(("i", "o", "g")):
        wp = np.concatenate([rows(wf1, which), rows(wb1, which)], axis=0)  # [32, 32]
        for b in range(4):
            w1_host[32 * b:32 * (b + 1), 32 * pi:32 * (pi + 1)] = wp.T
    w1_host = w1_host.astype(DT_NP)

    # fc: block-diagonal [128, 4]
    fcw_host = np.zeros((128, 4), np.float32)
    for b in range(4):
        fcw_host[32 * b:32 * (b + 1), b] = fc_w[0]
    fcw_host = fcw_host.astype(DT_NP)

    def brows(bvf, bvb, which):
        s = {"i": 0, "g": 2 * H, "o": 3 * H}[which]
        return np.concatenate([bvf[s:s + H], bvb[s:s + H]])

    bias_host = np.zeros((128, 8), np.float32)
    for col, (bvf, bvb, which) in enumerate((
        (bf0, bb0, "i"), (bf0, bb0, "o"), (bf0, bb0, "g"),
        (bf1, bb1, "i"), (bf1, bb1, "o"), (bf1, bb1, "g"),
    )):
        bias_host[:, col] = np.tile(brows(bvf, bvb, which), 4)
    bias_host[:, 6] = fc_b[0] + attn_b[0] * 0.0  # attn collapses; fc bias only
    return w0_host, w1_host, fcw_host, bias_host


_NC_CACHE = None
_LAST_IN_MAPS = None


def last_run_args():
    """For the local test harness: the (in_maps, nc) of the last kernel() call."""
    return _LAST_IN_MAPS, _NC_CACHE


def kernel(**inputs):
    global _NC_CACHE, _LAST_IN_MAPS
    x = np.ascontiguousarray(np.asarray(inputs["x"], dtype=np.float32))
    shared_names = ("wf0", "bf0", "wb0", "bb0", "wf1", "bf1", "wb1", "bb1",
                    "attn_w", "attn_b", "fc_w", "fc_b")
    shared = {k: np.asarray(inputs[k], dtype=np.float32) for k in shared_names}
    w0_host, w1_host, fcw_host, bias_host = _prep_shared(**shared)

    if _NC_CACHE is None:
        _NC_CACHE = build_kernel()
    nc = _NC_CACHE

    in_maps = []
    for c in range(N_CORES):
        xs = x[c * RC:(c + 1) * RC]  # [RC, D]
        # xt[p, k, r] = xs[r, 128k + p]
        xt = xs.reshape(RC, KCH, 128).transpose(2, 1, 0).astype(DT_NP)
        in_maps.append({
            "xt": np.ascontiguousarray(xt),
            "w0": w0_host, "w1": w1_host, "fcw": fcw_host, "bias": bias_host,
        })

    _LAST_IN_MAPS = in_maps
    res = run_bass_kernel_spmd(nc, in_maps, core_ids=list(range(N_CORES)))
    parts = []
    for c in range(N_CORES):
        o = res.results[c]["out"]  # [4, NGRP*BLK]
        parts.append(
            o.reshape(4, NGRP, BLK).transpose(1, 0, 2).reshape(RC)
        )
    y = np.concatenate(parts)
    return y.reshape(B, 1).astype(np.float32)


# revision 19
# speedup vs baseline: 1.0195x; 1.0195x over previous
"""Trainium2 Bass kernel for nn_AdvancedLSTMModel (B=262144, D=512, H=16).

The reference network collapses algebraically:
  - seq_len == 1 with zero initial state => LSTM cell is
      h = sigmoid(o) * tanh(sigmoid(i) * tanh(g)),  gates = x @ W_ih.T + b
    (forget gate f is computed but unused since c0 == 0)
  - softmax over a single timestep == 1, so attention context == h1
  - output = h1 @ fc_w.T + fc_b

Strategy: pure data parallel over 8 NeuronCores (batch sharded 32768 rows
per core). The host pre-transposes each x shard to feature-major layout
[128, 4, 32768] so the device streams x.T tiles directly as the matmul
moving operand (contraction over D on partitions; no on-device transpose).

On-device layout: batch is processed in groups of 2048 rows = 4 blocks of
512. All gate tensors are "block-packed": a [128, 512] tile whose partition
quarter q holds the 32 gate/feature channels of block q. Layer-0 gate
matmuls are 4-way column-tiled (M=32 per block into psum partitions 32q),
layer-1 matmuls are 4-way diagonal-tiled (K=32, M=32 at tile (32q, 32q)),
so ScalarE activations and VectorE multiplies always run on full 128
partitions.
"""

import sys

import numpy as np

import concourse.bass as bass
import concourse.mybir as mybir
import concourse.tile as tile
from concourse.bass_utils import run_bass_kernel_spmd

N_CORES = 8
B, D, H = 262144, 512, 16
RC = B // N_CORES          # rows per core
KCH = D // 128             # 4 contraction chunks
BLK = 512                  # rows per block (matmul moving N / psum bank)
GRP = 4 * BLK              # 2048 rows per group (4 column-tiled blocks)
NGRP = RC // GRP           # 16 groups per core
OUT_DMA_GROUPS = 4         # groups batched per output DMA

# Compute dtype for x / weights / activations (psum accumulation is fp32).
DT_NP = np.float16
DT_MB = mybir.dt.float16
F32 = mybir.dt.float32

AF = mybir.ActivationFunctionType


def _patched_drain_and_barrier(self, tick_clock, wait_clock):
    # The nix walrus only encodes one sync-wait per CTRL (drain) instruction;
    # split the Tile tail-drain's waits across one drain each.
    nc = self.nc
    from concourse.tile import ScopedClock

    drain_inst = nc.sync.drain()
    wait_clock.add_sem_waits(
        drain_inst.ins, ScopedClock({None: tick_clock.global_clock})
    )
    si = drain_inst.ins.sync_info
    if si is not None and si.on_wait and len(si.on_wait) > 1:
        waits = list(si.on_wait)
        si.on_wait = waits[:1]
        for w in waits[1:]:
            d2 = nc.sync.drain()
            d2.ins.sync_info = mybir.SyncInfo(on_wait=[w], on_update=[])
    nc.all_engine_barrier()
    popped = nc._tile_sem_poison_stack.pop()
    assert popped is self._sem_poison
    nc.clear_and_free_semaphores(list(self.sems.allocated().values()))
    nc.all_engine_barrier()


tile.TileContext._drain_and_barrier = _patched_drain_and_barrier

MAX_WAITS_PER_INST = 1


def _split_multi_waits(nc, limit=MAX_WAITS_PER_INST):
    """The nix walrus encodes at most `limit` sync-waits per instruction.
    Hoist excess waits onto preceding same-engine nops (engine queues are
    FIFO, so a nop-carried wait gates the next instruction identically)."""
    f = nc.m.functions[0]
    n_split = 0
    for bb in f.blocks:
        insts = bb.instructions
        out = []
        changed = False
        for inst in insts:
            si = inst.sync_info
            if si is not None and si.on_wait and len(si.on_wait) > limit:
                waits = list(si.on_wait)
                head, tail = waits[:-limit], waits[-limit:]
                for i in range(0, len(head), limit):
                    nop = mybir.InstNoOp(
                        name=nc.get_next_instruction_name(), ins=[], outs=[]
                    )
                    nop.engine = inst.engine
                    nop.sync_info = mybir.SyncInfo(
                        on_wait=head[i:i + limit], on_update=[]
                    )
                    out.append(nop)
                si.on_wait = tail
                changed = True
                n_split += 1
            out.append(inst)
        if changed:
            bb.instructions = out
    return n_split


def build_kernel():
    nc = bass.Bass()
    xt = nc.dram_tensor("xt", [128, KCH, RC], DT_MB, kind="ExternalInput")
    w0 = nc.dram_tensor("w0", [128, 3 * KCH * 128], DT_MB, kind="ExternalInput")
    w1 = nc.dram_tensor("w1", [128, 96], DT_MB, kind="ExternalInput")
    fcw = nc.dram_tensor("fcw", [128, 4], DT_MB, kind="ExternalInput")
    bias = nc.dram_tensor("bias", [128, 8], F32, kind="ExternalInput")
    out = nc.dram_tensor("out", [4, NGRP * BLK], F32, kind="ExternalOutput")

    with tile.TileContext(nc) as tc:
        with (
            tc.tile_pool(name="const", bufs=1) as cpool,
            tc.tile_pool(name="xin", bufs=3) as xpool,
            tc.tile_pool(name="work", bufs=3) as wpool,
            tc.tile_pool(name="outp", bufs=2) as opool,
            tc.tile_pool(name="ps0", bufs=1, space="PSUM") as psp0,
            tc.tile_pool(name="ps1", bufs=1, space="PSUM") as psp1,
            tc.tile_pool(name="ps_fc", bufs=2, space="PSUM") as ps_fc,
        ):
            # startup order: i-pass weights, then the first group's x chunks,
            # then everything else — so the first matmuls start ASAP
            w0_sb = cpool.tile([128, 3 * KCH * 128], DT_MB)
            nc.sync.dma_start(out=w0_sb[:, 0:KCH * 128], in_=w0[:, 0:KCH * 128])
            xg_first = xpool.tile([128, KCH, GRP], DT_MB, tag="xg")
            for k in range(KCH):
                nc.sync.dma_start(out=xg_first[:, k, :], in_=xt[:, k, 0:GRP])
            nc.sync.dma_start(out=w0_sb[:, KCH * 128:3 * KCH * 128],
                              in_=w0[:, KCH * 128:3 * KCH * 128])
            bias_sb = cpool.tile([128, 8], F32)
            nc.sync.dma_start(out=bias_sb[:], in_=bias[:])
            w1_sb = cpool.tile([128, 96], DT_MB)
            nc.sync.dma_start(out=w1_sb[:], in_=w1[:])
            fcw_sb = cpool.tile([128, 4], DT_MB)
            nc.sync.dma_start(out=fcw_sb[:], in_=fcw[:])

            # two-stage software pipeline over groups: stage B (layer 1, fc,
            # out) for group t-1 is emitted before stage A (x DMA, layer 0)
            # for group t, so every engine's FIFO leads with ready work.
            stash = {}
            ob = None

            def stage_a_mm(g):
                if g == 0:
                    xg = xg_first
                else:
                    xg = xpool.tile([128, KCH, GRP], DT_MB, tag="xg")
                    nc.sync.dma_start(out=xg[:],
                                      in_=xt[:, :, g * GRP:(g + 1) * GRP])
                ps = []
                for pi in range(3):  # 0: i, 1: o, 2: g
                    p = psp0.tile([128, BLK], F32, tag=f"l0p{pi}")
                    for k in range(KCH):
                        for b in range(4):
                            off = (pi * KCH + k) * 128 + 32 * b
                            nc.tensor.matmul(
                                p[32 * b:32 * b + 32, :],
                                lhsT=w0_sb[:, off:off + 32],
                                rhs=xg[:, k, BLK * b:BLK * (b + 1)],
                                start=(k == 0),
                                stop=(k == KCH - 1),
                                tile_position=(0, 32 * b),
                            )
                    ps.append(p)
                stash[("ps0", g)] = ps

            def stage_a_act(g):
                ps = stash.pop(("ps0", g))
                io0 = wpool.tile([128, 2 * BLK], DT_MB, tag="io0")
                g0 = wpool.tile([128, BLK], DT_MB, tag="g0")
                # order: sigma(i), tanh(g) first so the DVE c-mul can run
                # under sigma(o); then tanh(c) is ready right on time.
                nc.scalar.activation(io0[:, 0:BLK], ps[0][:], AF.Sigmoid,
                                     bias=bias_sb[:, 0:1])
                nc.scalar.activation(g0[:], ps[2][:], AF.Tanh,
                                     bias=bias_sb[:, 2:3])
                # c and tanh(c) overwrite the sigma(i)/tanh(g) slots in place:
                # fewer tiles -> fewer release semaphores
                nc.vector.tensor_mul(io0[:, 0:BLK], io0[:, 0:BLK], g0[:])
                nc.scalar.activation(io0[:, BLK:2 * BLK], ps[1][:], AF.Sigmoid,
                                     bias=bias_sb[:, 1:2])
                nc.scalar.activation(g0[:], io0[:, 0:BLK], AF.Tanh,
                                     bias=bias_sb[:, 7:8])
                h0 = wpool.tile([128, BLK], DT_MB, tag="h0")
                nc.vector.tensor_mul(h0[:], io0[:, BLK:2 * BLK], g0[:])
                stash[("h0", g)] = h0

            def stage_b_mm(g):
                h0 = stash.pop(("h0", g))
                ps = []
                for pi in range(3):
                    p = psp1.tile([128, BLK], F32, tag=f"l1p{pi}")
                    for b in range(4):
                        nc.tensor.matmul(
                            p[32 * b:32 * b + 32, :],
                            lhsT=w1_sb[32 * b:32 * b + 32, 32 * pi:32 * pi + 32],
                            rhs=h0[32 * b:32 * b + 32, :],
                            start=True,
                            stop=True,
                            tile_position=(32 * b, 32 * b),
                        )
                    ps.append(p)
                stash[("ps1", g)] = ps

            def stage_b_act(g):
                ps = stash.pop(("ps1", g))
                io1 = wpool.tile([128, 2 * BLK], DT_MB, tag="io1")
                g1 = wpool.tile([128, BLK], DT_MB, tag="g1")
                nc.scalar.activation(io1[:, 0:BLK], ps[0][:], AF.Sigmoid,
                                     bias=bias_sb[:, 3:4])
                nc.scalar.activation(g1[:], ps[2][:], AF.Tanh,
                                     bias=bias_sb[:, 5:6])
                nc.vector.tensor_mul(io1[:, 0:BLK], io1[:, 0:BLK], g1[:])
                nc.scalar.activation(io1[:, BLK:2 * BLK], ps[1][:], AF.Sigmoid,
                                     bias=bias_sb[:, 4:5])
                nc.scalar.activation(g1[:], io1[:, 0:BLK], AF.Tanh,
                                     bias=bias_sb[:, 7:8])
                h1 = wpool.tile([128, BLK], DT_MB, tag="h1")
                nc.vector.tensor_mul(h1[:], io1[:, BLK:2 * BLK], g1[:])
                stash[("h1", g)] = h1

            def stage_fc(g):
                nonlocal ob
                h1 = stash.pop(("h1", g))
                pf = ps_fc.tile([4, BLK], F32, tag="fc")
                nc.tensor.matmul(pf[:], lhsT=fcw_sb[:, 0:4], rhs=h1[:],
                                 start=True, stop=True, tile_position=(0, 0))
                if g % OUT_DMA_GROUPS == 0:
                    ob = opool.tile([4, OUT_DMA_GROUPS * BLK], F32, tag="ob")
                go = g % OUT_DMA_GROUPS
                # fc bias-add + psum evacuation on the (idle) vector engine
                nc.vector.tensor_scalar_add(ob[:, go * BLK:(go + 1) * BLK],
                                            pf[:], bias_sb[0:4, 6:7])
                if go == OUT_DMA_GROUPS - 1:
                    j = g // OUT_DMA_GROUPS
                    w = OUT_DMA_GROUPS * BLK
                    nc.sync.dma_start(out=out[:, j * w:(j + 1) * w], in_=ob[:])

            # slot t: B-MM(t-1) | A-acts(t) | A-MM(t+1) | B-acts(t-1) | fc(t-1)
            # A-acts(t) lead the ACT FIFO already-ready (their matmuls ran
            # last slot); by the time they finish, B-MM(t-1) has long drained
            # so B-acts(t-1) follow without a gap. The fc matmul (gated on the
            # full act chain) sits last in the PE FIFO behind ready work.
            stage_a_mm(0)
            for t in range(0, NGRP + 1):
                if t >= 1:
                    stage_b_mm(t - 1)
                if t < NGRP:
                    stage_a_act(t)
                if t + 1 < NGRP:
                    stage_a_mm(t + 1)
                if t >= 1:
                    stage_b_act(t - 1)
                    stage_fc(t - 1)
    _split_multi_waits(nc)
    return nc


def _prep_shared(wf0, bf0, wb0, bb0, wf1, bf1, wb1, bb1, attn_w, attn_b,
                 fc_w, fc_b):
    """Build the replicated weight/bias arrays in device layout."""
    # torch LSTM gate row order within [4H]: i, f, g, o
    def rows(w, which):
        s = {"i": 0, "g": 2 * H, "o": 3 * H}[which]
        return w[s:s + H]

    # layer 0 stationary: [128(d), 3(pass), KCH, 128(4 x 32 dup)]
    w0_host = np.zeros((128, 3, KCH, 128), np.float32)
    for pi, which in enumerate(("i", "o", "g")):
        wp = np.concatenate([rows(wf0, which), rows(wb0, which)], axis=0)  # [32, D]
        for k in range(KCH):
            blk = wp[:, 128 * k:128 * (k + 1)].T  # [128(d), 32]
            for b in range(4):
                w0_host[:, pi, k, 32 * b:32 * (b + 1)] = blk
    w0_host = w0_host.reshape(128, 3 * KCH * 128).astype(DT_NP)

    # layer 1 stationary: [128(p = 32-dup'd input), 3*32(out)]
    w1_host = np.zeros((128, 96), np.float32)
    for pi, which in enumerate(("i", "o", "g")):
        wp = np.concatenate([rows(wf1, which), rows(wb1, which)], axis=0)  # [32, 32]
        for b in range(4):
            w1_host[32 * b:32 * (b + 1), 32 * pi:32 * (pi + 1)] = wp.T
    w1_host = w1_host.astype(DT_NP)

    # fc: block-diagonal [128, 4]
    fcw_host = np.zeros((128, 4), np.float32)
    for b in range(4):
        fcw_host[32 * b:32 * (b + 1), b] = fc_w[0]
    fcw_host = fcw_host.astype(DT_NP)

    def brows(bvf, bvb, which):
        s = {"i": 0, "g": 2 * H, "o": 3 * H}[which]
        return np.concatenate([bvf[s:s + H], bvb[s:s + H]])

    bias_host = np.zeros((128, 8), np.float32)
    for col, (bvf, bvb, which) in enumerate((
        (bf0, bb0, "i"), (bf0, bb0, "o"), (bf0, bb0, "g"),
        (bf1, bb1, "i"), (bf1, bb1, "o"), (bf1, bb1, "g"),
    )):
        bias_host[:, col] = np.tile(brows(bvf, bvb, which), 4)
    bias_host[:, 6] = fc_b[0] + attn_b[0] * 0.0  # attn collapses; fc bias only
    return w0_host, w1_host, fcw_host, bias_host


_NC_CACHE = None
_LAST_IN_MAPS = None


def last_run_args():
    """For the local test harness: the (in_maps, nc) of the last kernel() call."""
    return _LAST_IN_MAPS, _NC_CACHE


def kernel(**inputs):
    global _NC_CACHE, _LAST_IN_MAPS
    x = np.ascontiguousarray(np.asarray(inputs["x"], dtype=np.float32))
    shared_names = ("wf0", "bf0", "wb0", "bb0", "wf1", "bf1", "wb1", "bb1",
                    "attn_w", "attn_b", "fc_w", "fc_b")
    shared = {k: np.asarray(inputs[k], dtype=np.float32) for k in shared_names}
    w0_host, w1_host, fcw_host, bias_host = _prep_shared(**shared)

    if _NC_CACHE is None:
        _NC_CACHE = build_kernel()
    nc = _NC_CACHE

    in_maps = []
    for c in range(N_CORES):
        xs = x[c * RC:(c + 1) * RC]  # [RC, D]
        # xt[p, k, r] = xs[r, 128k + p]
        xt = xs.reshape(RC, KCH, 128).transpose(2, 1, 0).astype(DT_NP)
        in_maps.append({
            "xt": np.ascontiguousarray(xt),
            "w0": w0_host, "w1": w1_host, "fcw": fcw_host, "bias": bias_host,
        })

    _LAST_IN_MAPS = in_maps
    res = run_bass_kernel_spmd(nc, in_maps, core_ids=list(range(N_CORES)))
    parts = []
    for c in range(N_CORES):
        o = res.results[c]["out"]  # [4, NGRP*BLK]
        parts.append(
            o.reshape(4, NGRP, BLK).transpose(1, 0, 2).reshape(RC)
        )
    y = np.concatenate(parts)
    return y.reshape(B, 1).astype(np.float32)
